# revision 2
# baseline (speedup 1.0000x reference)
"""GCNCheb Trainium2 kernel: out[b,n,fo] = sum_k T_k[b,n,:] @ W[k] + bias.

T_k recurrence (matrix powers P_j = L^j x with T0=P0, T1=P1, Tk=2*P_k - T_{k-2})
is linear, so the K/F_in contraction is re-expressed over pure powers with
host-precombined weights V_j:
    out = P0 (W0-W2) + P1 (W1-W3) + P2 (2 W2) + P3 (2 W3) + bias

Distribution over 8 NeuronCores: 1D row-shard of L. Core r holds the column
slice Lc_r = L[:, r*1024:(r+1)*1024] (== L[rows_r,:].T since L is symmetric),
pre-tiled on host to [4, 128, 64, 256] (q quarters of the 1024 columns), bf16,
fully SBUF-resident (16 MB). X is [N, B*F_in] = [8192, 128] (batch folded into
columns), pre-tiled to [128, 64, 128].

Schedule (all times approximate, derived from the baseline trace):
- L + X stream in over BOTH HWDGE queues (sync: X-even + q0 then q2;
  scalar: X-odd + q1 then q3), interleaved in consumption order, so step 1's
  two output halves (mt 0-3 / 4-7) complete as their L quarters land.
- A tiny warmup AllGather fires at t~2us to absorb collective-firmware
  startup cost before the first real gather.
- Each step boundary does TWO AllGathers (half shards, 128KB in / 1MB out),
  since the CC core serializes collectives ~10-14us apart regardless of size;
  the half granularity lets the consuming step start on the first half while
  the second gathers. Scatter-back DMAs are split across both queues.
- Steps 2 consumes k-tiles in gather-arrival order with ALL 8 PSUM banks
  accumulating concurrently (kt-outer), staged so the first output half
  (mt 0-3) finishes its accumulation first and gathers ~7us earlier.
- Step 3 computes P3^T directly via the symmetric-L trick
  (pt3 = X2^T L, free dim 512), consuming k-tiles in arrival order with both
  ns-chains interleaved (stationary operand reuse), then projects on-device.
Final output is assembled host-side (adds bias); no output collective.
"""

import os
import sys

sys.path.insert(0, "/opt/trn_rl_repo")

import numpy as np

import concourse.bass as bass
import concourse.mybir as mybir
import concourse.tile as tile
from concourse import bacc, bass_utils
from concourse.masks import make_identity

B, N, F_IN, F_OUT, K = 4, 8192, 32, 64, 4
NCORES = 8
P = 128
SH = N // NCORES          # rows per core (1024)
SH4 = SH // 4             # L quarter columns (256)
BF = B * F_IN             # folded X columns (128)
KT = N // P               # contraction tiles (64)
MT = SH // P              # output row tiles per core (8)
MH = MT // 2              # half-shard m-tiles (4)
QH = 2                    # output halves: (b in {2h, 2h+1}) x F_OUT = 128 partitions

VARIANT = os.environ.get("GCN_VARIANT", "bf16")


def _np_dt(variant="bf16"):
    import ml_dtypes

    return np.dtype(ml_dtypes.bfloat16)


def build_nc(variant="bf16"):
    dt = mybir.dt.bfloat16
    f32 = mybir.dt.float32
    groups = [list(range(NCORES))]

    nc = bacc.Bacc()
    # all pre-tiled on host: partition-major, fully contiguous per partition
    Lc = nc.dram_tensor("Lc", [4, P, KT, SH4], dt, kind="ExternalInput")
    X0 = nc.dram_tensor("X0", [P, KT, BF], dt, kind="ExternalInput")
    X0T = nc.dram_tensor("X0T", [BF, SH], dt, kind="ExternalInput")
    WH = nc.dram_tensor("WH", [K, QH, BF, P], dt, kind="ExternalInput")
    OUT = nc.dram_tensor("OUT", [QH, P, SH], f32, kind="ExternalOutput")

    # k-tile arrival order after a half-gather of mts [mt0, mt0+nmt)
    def kts_of(mt0, nmt):
        return [r * MT + mt0 + m for r in range(NCORES) for m in range(nmt)]

    ktA = kts_of(0, MH)
    ktB = kts_of(MH, MH)

    with tile.TileContext(nc) as tc:
        with (
            tc.tile_pool(name="lres", bufs=1) as lres_pool,
            tc.tile_pool(name="xbuf", bufs=2) as x_pool,
            tc.tile_pool(name="ybuf", bufs=2) as y_pool,
            tc.tile_pool(name="proj", bufs=1) as proj_pool,
            tc.tile_pool(name="psum", bufs=1, space="PSUM") as psum_pool,
            tc.tile_pool(name="dram", bufs=1, space="DRAM") as dram_pool,
        ):
            # --- identity (transposes + warmup payload), warmup collective ---
            ident = proj_pool.tile([P, P], dt, tag="ident")
            make_identity(nc, ident[:])
            win = dram_pool.tile([1, 64], dt, name="win")
            wout = dram_pool.tile(
                [NCORES, 64], dt, addr_space="Shared", name="wout"
            )
            nc.sync.dma_start(win.opt(), ident[0:1, 0:64])
            nc.gpsimd.collective_compute(
                "AllGather",
                mybir.AluOpType.bypass,
                replica_groups=groups,
                ins=[win.opt()],
                outs=[wout.opt()],
            )

            # --- initial loads, interleaved in consumption order ---
            # sync:   X[even 8-blocks], Lq0 blocks, then Lq2
            # scalar: whs, pt0, X[odd 8-blocks], Lq1 blocks, then Lq3
            x_cur = x_pool.tile([P, KT, BF], dt, tag="x", name="x0")
            lc_res = lres_pool.tile([P, 4, KT, SH4], dt, tag="lc_res")

            whs = proj_pool.tile([P, K, QH, P], dt, tag="whs")
            nc.scalar.dma_start(whs[:], WH.rearrange("k h p m -> p k h m"))
            pt0 = proj_pool.tile([P, SH], dt, tag="pt0")
            nc.scalar.dma_start(pt0[:], X0T[:, :])

            # leading small chunks so the very first matmuls start early
            nc.sync.dma_start(x_cur[:, 0:2, :], X0[:, 0:2, :])
            nc.sync.dma_start(lc_res[:, 0, 0:2, :], Lc[0, :, 0:2, :])
            nc.scalar.dma_start(lc_res[:, 1, 0:2, :], Lc[1, :, 0:2, :])
            nc.sync.dma_start(x_cur[:, 2:8, :], X0[:, 2:8, :])
            nc.sync.dma_start(lc_res[:, 0, 2:8, :], Lc[0, :, 2:8, :])
            nc.scalar.dma_start(lc_res[:, 1, 2:8, :], Lc[1, :, 2:8, :])
            for ko in range(8, KT, 8):
                xq = nc.sync if (ko // 8) % 2 == 0 else nc.scalar
                xq.dma_start(x_cur[:, ko : ko + 8, :], X0[:, ko : ko + 8, :])
                nc.sync.dma_start(
                    lc_res[:, 0, ko : ko + 8, :], Lc[0, :, ko : ko + 8, :]
                )
                nc.scalar.dma_start(
                    lc_res[:, 1, ko : ko + 8, :], Lc[1, :, ko : ko + 8, :]
                )
            for ko in range(0, KT, 8):
                nc.sync.dma_start(
                    lc_res[:, 2, ko : ko + 8, :], Lc[2, :, ko : ko + 8, :]
                )
                nc.scalar.dma_start(
                    lc_res[:, 3, ko : ko + 8, :], Lc[3, :, ko : ko + 8, :]
                )

            pt = [pt0, None, None, None]
            out_sb = proj_pool.tile([P, QH, 2, 512], f32, tag="out_sb")

            def lhsT_res(kt, mt):
                q, m = divmod(mt, 2)
                return lc_res[:, q, kt, m * P : (m + 1) * P]

            def gather_half(step, mt0, yshd, x_nxt):
                """AllGather the mt-half [mt0, mt0+MH) of the step's shard and
                scatter it back into x_nxt across both DMA queues."""
                shard = dram_pool.tile([P, MH, BF], dt, name=f"sh{step}_{mt0}")
                full = dram_pool.tile(
                    [NCORES * P, MH, BF],
                    dt,
                    addr_space="Shared",
                    name=f"fl{step}_{mt0}",
                )
                nc.sync.dma_start(shard.opt(), yshd[:, mt0 : mt0 + MH, :])
                nc.gpsimd.collective_compute(
                    "AllGather",
                    mybir.AluOpType.bypass,
                    replica_groups=groups,
                    ins=[shard.opt()],
                    outs=[full.opt()],
                )
                xv = x_nxt[:].rearrange("p (r mt) f -> p r mt f", r=NCORES)
                fv = full[:].rearrange("(r p) mt f -> p r mt f", p=P)
                nc.scalar.dma_start(
                    xv[:, 0:4, mt0 : mt0 + MH, :], fv[:, 0:4, :, :]
                )
                nc.sync.dma_start(
                    xv[:, 4:8, mt0 : mt0 + MH, :], fv[:, 4:8, :, :]
                )

            def transposes(step, yshd, mts):
                """PE-transpose the row shard into P_j^T [BF, SH] for the
                projection."""
                if pt[step] is None:
                    pt[step] = proj_pool.tile(
                        [P, SH], dt, tag=f"pt{step}", name=f"pt{step}"
                    )
                for mt in mts:
                    tp = psum_pool.tile(
                        [P, P], dt, tag=f"ps{mt}", name=f"tp{step}_{mt}"
                    )
                    nc.tensor.transpose(tp[:], yshd[:, mt, :], ident[:])
                    nc.vector.tensor_copy(
                        pt[step][:, mt * P : (mt + 1) * P], tp[:]
                    )

            # --- step 1: phase-outer (L arrives q-major), halves gathered ---
            y1 = y_pool.tile([P, MT, BF], dt, tag="yshd", name="y1")
            x1 = x_pool.tile([P, KT, BF], dt, tag="x", name="x1")
            yp1 = {
                mt: psum_pool.tile([P, BF], f32, tag=f"ps{mt}", name=f"y1_{mt}")
                for mt in range(MT)
            }
            for mt0 in (0, MH):
                mts = range(mt0, mt0 + MH)
                for kt in range(KT):
                    for mt in mts:
                        nc.tensor.matmul(
                            yp1[mt][:],
                            lhsT=lhsT_res(kt, mt),
                            rhs=x_cur[:, kt, :],
                            start=(kt == 0),
                            stop=(kt == KT - 1),
                        )
                for mt in mts:
                    nc.vector.tensor_copy(y1[:, mt, :], yp1[mt][:])
                gather_half(1, mt0, y1, x1)
                transposes(1, y1, mts)

            # --- step 2: kt-outer in arrival order, all 8 banks accumulate;
            # output halves staged so mts 0-3 finish (and gather) first ---
            y2 = y_pool.tile([P, MT, BF], dt, tag="yshd", name="y2")
            x2 = x_pool.tile([P, KT, BF], dt, tag="x", name="x2")
            yp2 = {
                mt: psum_pool.tile([P, BF], f32, tag=f"ps{mt}", name=f"y2_{mt}")
                for mt in range(MT)
            }

            def step2_mms(kts, mts, start, stop):
                for ki, kt in enumerate(kts):
                    for mt in mts:
                        nc.tensor.matmul(
                            yp2[mt][:],
                            lhsT=lhsT_res(kt, mt),
                            rhs=x1[:, kt, :],
                            start=(start and ki == 0),
                            stop=(stop and ki == len(kts) - 1),
                        )

            step2_mms(ktA, range(0, MH), True, False)
            step2_mms(ktA, range(MH, MT), True, False)
            step2_mms(ktB, range(0, MH), False, True)
            for mt in range(0, MH):
                nc.vector.tensor_copy(y2[:, mt, :], yp2[mt][:])
            gather_half(2, 0, y2, x2)
            step2_mms(ktB, range(MH, MT), False, True)
            for mt in range(MH, MT):
                nc.vector.tensor_copy(y2[:, mt, :], yp2[mt][:])
            gather_half(2, MH, y2, x2)
            transposes(2, y2, range(MT))

            # --- step 3: pt3 = X2^T L directly (free dim 512), both ns
            # chains interleaved per kt for stationary reuse ---
            pt3 = proj_pool.tile([P, SH], dt, tag="pt3", name="pt3")
            pt[3] = pt3
            pp3 = {
                ns: psum_pool.tile(
                    [P, 512], f32, tag=f"ps{4 * ns}", name=f"p3_{ns}"
                )
                for ns in range(2)
            }
            order = ktA + ktB
            for ki, kt in enumerate(order):
                for ns in range(2):
                    nc.tensor.matmul(
                        pp3[ns][:],
                        lhsT=x2[:, kt, :],
                        rhs=lc_res[:, 2 * ns : 2 * ns + 2, kt, :],
                        start=(ki == 0),
                        stop=(ki == len(order) - 1),
                    )

            # --- projection + output (per ns half) ---
            for ns in range(2):
                nc.vector.tensor_copy(
                    pt3[:, ns * 512 : (ns + 1) * 512], pp3[ns][:]
                )
                for h in range(QH):
                    pp = psum_pool.tile(
                        [P, 512],
                        f32,
                        tag=f"ps{4 * ns + 1 + h}",
                        name=f"pp{h}_{ns}",
                    )
                    for j in range(K):
                        nc.tensor.matmul(
                            pp[:],
                            lhsT=whs[:, j, h, :],
                            rhs=pt[j][:, ns * 512 : (ns + 1) * 512],
                            start=(j == 0),
                            stop=(j == K - 1),
                        )
                    nc.vector.tensor_copy(out_sb[:, h, ns, :], pp[:])
                eng = nc.sync if ns == 0 else nc.scalar
                eng.dma_start(
                    OUT.rearrange("h q (s n) -> q h s n", s=2)[:, :, ns, :],
                    out_sb[:, :, ns, :],
                )

    nc.compile()
    return nc


_CACHED = {}


def _get_nc(variant=VARIANT):
    if "nc" not in _CACHED:
        _CACHED["nc"] = build_nc()
    return _CACHED["nc"]


def _prep_inputs(x, L, weight, variant=VARIANT):
    np_dt = _np_dt(variant)
    f32 = np.float32

    X0 = np.ascontiguousarray(
        x.astype(f32).transpose(1, 0, 2).reshape(N, BF)
    )  # [N, (b,fi)]
    X0_t = np.ascontiguousarray(
        X0.reshape(KT, P, BF).transpose(1, 0, 2)
    ).astype(np_dt)  # [P, KT, BF]
    W = weight.astype(f32)
    V = np.stack(
        [W[0] - W[2], W[1] - W[3], 2.0 * W[2], 2.0 * W[3]]
    )  # [4, F_IN, F_OUT]
    # block-diagonal packing: WH[j, h, b*F_IN+fi, bl*F_OUT+fo] = V[j,fi,fo]
    # for b == 2h + bl
    WH = np.zeros((K, QH, BF, P), dtype=f32)
    for j in range(K):
        for b in range(B):
            h, bl = divmod(b, 2)
            WH[j, h, b * F_IN : (b + 1) * F_IN, bl * F_OUT : (bl + 1) * F_OUT] = V[j]
    WH = WH.astype(np_dt)

    in_maps = []
    for r in range(NCORES):
        rows = slice(r * SH, (r + 1) * SH)
        Lc_r = np.ascontiguousarray(
            L[:, rows].reshape(KT, P, 4, SH4).transpose(2, 1, 0, 3)
        ).astype(np_dt)  # [4, P, KT, SH4]
        X0T_r = np.ascontiguousarray(X0[rows, :].T).astype(np_dt)
        in_maps.append({"Lc": Lc_r, "X0": X0_t, "X0T": X0T_r, "WH": WH})
    return in_maps


def _assemble(results, bias):
    out = np.empty((B, N, F_OUT), dtype=np.float32)
    for r in range(NCORES):
        outT = results[r]["OUT"]  # [QH, 128, SH]
        for b in range(B):
            h, bl = divmod(b, 2)
            out[b, r * SH : (r + 1) * SH, :] = outT[
                h, bl * F_OUT : (bl + 1) * F_OUT, :
            ].T
    out += bias.astype(np.float32)
    return out


def run(x, L, weight, bias, variant=VARIANT, trace=False):
    nc = _get_nc(variant)
    in_maps = _prep_inputs(x, L, weight, variant)
    last_err = None
    for attempt in range(3):
        try:
            res = bass_utils.run_bass_kernel_spmd(
                nc,
                in_maps,
                core_ids=list(range(NCORES)),
                trace=trace,
                trace_cores=list(range(NCORES)) if trace else None,
            )
            break
        except Exception as e:  # transient device wedge: retry
            last_err = e
            import time

            time.sleep(10)
    else:
        raise last_err
    out = _assemble(res.results, bias)
    return out, res


def kernel(x, L, weight, bias):
    out, _ = run(
        np.asarray(x), np.asarray(L), np.asarray(weight), np.asarray(bias)
    )
    return out
